# revision 4
# baseline (speedup 1.0000x reference)
import numpy as np

# nn_MultiHeadedAttention: B=4, S=2048, D_MODEL=1024, H=16, D_K=64, fp32.
# Sharding: 8 cores = 4 batches x 2 head-groups (8 heads each).
# Per-core: head-group projections (fp32r matmuls), scores computed
# transposed ST[k,q], exp on ACT straight out of 2-bank PSUM reads,
# PV with an appended ones-column so the softmax denominator falls out
# of row 64 of the PV accumulation, PE-broadcast reciprocal, out-proj
# partial y; host sums the two head-group partials and adds bo.

B, S, D, H, DK = 4, 2048, 1024, 16, 64
NCORES = 8
DG = 512  # dims per head-group (8 heads x 64)

_NC_CACHE = {}
LAST_EXEC_NS = None


def _build_nc():
    import concourse.bacc as bacc
    import concourse.tile as tile
    from concourse import mybir

    F32R = mybir.dt.float32r
    F32 = mybir.dt.float32
    EXP = mybir.ActivationFunctionType.Exp

    nc = bacc.Bacc(None, target_bir_lowering=False, debug=True)

    xqT = nc.dram_tensor("xqT", [D, S], F32R, kind="ExternalInput")
    xkT = nc.dram_tensor("xkT", [D, S], F32R, kind="ExternalInput")
    xvT = nc.dram_tensor("xvT", [D, S], F32R, kind="ExternalInput")
    wqT = nc.dram_tensor("wqT", [D, DG], F32R, kind="ExternalInput")
    wkT = nc.dram_tensor("wkT", [D, DG], F32R, kind="ExternalInput")
    wvT = nc.dram_tensor("wvT", [D, DG], F32R, kind="ExternalInput")
    woT = nc.dram_tensor("woT", [DG, D], F32R, kind="ExternalInput")
    bqc = nc.dram_tensor("bqc", [128, 4], F32, kind="ExternalInput")
    bkc = nc.dram_tensor("bkc", [128, 4], F32, kind="ExternalInput")
    bvr = nc.dram_tensor("bvr", [DG], F32, kind="ExternalInput")
    y_d = nc.dram_tensor("y", [S, D], F32R, kind="ExternalOutput")

    import concourse.bass as bass

    with (
        tile.TileContext(nc) as tc,
        nc.allow_low_precision(reason="float32r carries full fp32 bits"),
        tc.tile_pool(name="persist", bufs=1) as persist,
    ):
        QT = [persist.tile([128, S], F32R, name=f"QT{p}") for p in range(4)]
        KT = [persist.tile([128, S], F32R, name=f"KT{p}") for p in range(4)]
        AT = [persist.tile([128, S], F32R, name=f"AT{p}") for p in range(4)]
        vones = [persist.tile([128, 8, 65], F32R, name=f"vones{s}") for s in range(16)]
        bq_sb = persist.tile([128, 4], F32, name="bq_sb")
        bk_sb = persist.tile([128, 4], F32, name="bk_sb")
        bv_sb = persist.tile([128, DG], F32, name="bv_sb")
        ones_row = persist.tile([1, 64], F32R, name="ones_row")

        nc.gpsimd.dma_start(bq_sb[:], bqc[:])
        nc.gpsimd.dma_start(bk_sb[:], bkc[:])
        bv_ap = bvr[:]
        bv_bcast = bass.AP(tensor=bv_ap.tensor, offset=bv_ap.offset, ap=[[0, 128], *bv_ap.ap])
        nc.gpsimd.dma_start(bv_sb[:], bv_bcast)
        nc.vector.memset(ones_row[:].bitcast(F32), 1.0)
        for s in range(16):
            nc.vector.memset(vones[s][:, :, 64:65].bitcast(F32), 1.0)

        # ---- Q then K projections ----
        for (x_d, w_d, b_sb, OUT) in ((xqT, wqT, bq_sb, QT), (xkT, wkT, bk_sb, KT)):
            with (
                tc.tile_pool(name="projqk", bufs=2) as stage,
                tc.tile_pool(name="psprojqk", bufs=4, space="PSUM") as psp,
            ):
                wt = stage.tile([128, 8, DG], F32R, name="wt", bufs=1)
                for i in range(8):
                    nc.gpsimd.dma_start(wt[:, i, :], w_d[i * 128 : (i + 1) * 128, :])
                for qc in range(4):
                    qs = slice(qc * 512, (qc + 1) * 512)
                    xs = stage.tile([128, 8, 512], F32R, name="xs")
                    for i in range(8):
                        nc.sync.dma_start(xs[:, i, :], x_d[i * 128 : (i + 1) * 128, qs])
                    for p in range(4):
                        pp = psp.tile([128, 512], F32, name="pp")
                        for i in range(8):
                            nc.tensor.matmul(
                                pp[:],
                                wt[:, i, p * 128 : (p + 1) * 128],
                                xs[:, i, :],
                                start=(i == 0),
                                stop=(i == 7),
                            )
                        nc.vector.tensor_scalar_add(OUT[p][:, qs], pp[:], b_sb[:, p : p + 1])

        # ---- V projection (natural layout [s, d] + ones column) ----
        with (
            tc.tile_pool(name="projv", bufs=3) as stage,
            tc.tile_pool(name="psprojv", bufs=4, space="PSUM") as psp,
        ):
            wv = stage.tile([128, 8, DG], F32R, name="wv", bufs=1)
            for i in range(8):
                nc.gpsimd.dma_start(wv[:, i, :], wvT[i * 128 : (i + 1) * 128, :])
            for sb in range(16):
                ss = slice(sb * 128, (sb + 1) * 128)
                xv = stage.tile([128, 8, 128], F32R, name="xv")
                for i in range(8):
                    nc.sync.dma_start(xv[:, i, :], xvT[i * 128 : (i + 1) * 128, ss])
                vp = psp.tile([128, DG], F32, name="vp")
                for i in range(8):
                    nc.tensor.matmul(
                        vp[:], xv[:, i, :], wv[:, i, :], start=(i == 0), stop=(i == 7)
                    )
                nc.vector.tensor_add(
                    vones[sb][:, :, 0:64],
                    vp[:].rearrange("p (h d) -> p h d", h=8),
                    bv_sb[:].rearrange("p (h d) -> p h d", h=8),
                )

        # ---- attention: scores -> exp -> PV(+denom) -> normalize ----
        with (
            tc.tile_pool(name="attn_sb", bufs=3) as asb,
            tc.tile_pool(name="ps_st", bufs=2, space="PSUM") as ps_st,
            tc.tile_pool(name="ps_pv", bufs=2, space="PSUM") as ps_pv,
            tc.tile_pool(name="ps_bc", bufs=2, space="PSUM") as ps_bc,
        ):
            for p in range(4):
                for hl in range(2):
                    hb = hl * 64
                    hidx = p * 2 + hl
                    for qc in range(4):
                        qs = slice(qc * 512, (qc + 1) * 512)
                        pv = ps_pv.tile([128, 512], F32, name="pv")
                        for kg in range(8):
                            st = ps_st.tile([128, 2, 512], F32, name="st")
                            for j in range(2):
                                kb = kg * 2 + j
                                nc.tensor.matmul(
                                    st[:, j, :],
                                    KT[p][hb : hb + 64, kb * 128 : (kb + 1) * 128],
                                    QT[p][hb : hb + 64, qs],
                                    start=True,
                                    stop=True,
                                )
                            et = asb.tile([128, 2, 512], F32R, name="et")
                            nc.scalar.activation(out=et[:], in_=st[:], func=EXP, scale=0.125)
                            for j in range(2):
                                kb = kg * 2 + j
                                nc.tensor.matmul(
                                    pv[0:65, :],
                                    vones[kb][:, hidx, :],
                                    et[:, j, :],
                                    start=(kb == 0),
                                    stop=(kb == 15),
                                )
                        recip = asb.tile([1, 512], F32R, name="recip")
                        nc.vector.reciprocal(recip[:], pv[64:65, :])
                        bc = ps_bc.tile([128, 512], F32, name="bc")
                        nc.tensor.matmul(
                            bc[0:64, :], ones_row[:], recip[:], start=True, stop=True
                        )
                        bcs = asb.tile([64, 512], F32R, name="bcs")
                        nc.vector.tensor_copy(bcs[:], bc[0:64, :])
                        nc.vector.tensor_mul(AT[p][hb : hb + 64, qs], pv[0:64, :], bcs[:])

        # ---- output projection (partial y for this head-group) ----
        with (
            tc.tile_pool(name="out_sb", bufs=1) as osb,
            tc.tile_pool(name="out_y", bufs=3) as oy,
            tc.tile_pool(name="ps_y", bufs=4, space="PSUM") as ps_y,
        ):
            wo = osb.tile([128, 4, D], F32R, name="wo")
            for p in range(4):
                nc.gpsimd.dma_start(wo[:, p, :], woT[p * 128 : (p + 1) * 128, :])
            for sb in range(16):
                ss = slice(sb * 128, (sb + 1) * 128)
                ys = oy.tile([128, 2, 512], F32R, name="ys")
                for oc in range(2):
                    yp = ps_y.tile([128, 512], F32, name="yp")
                    for p in range(4):
                        nc.tensor.matmul(
                            yp[:],
                            AT[p][:, ss],
                            wo[:, p, oc * 512 : (oc + 1) * 512],
                            start=(p == 0),
                            stop=(p == 3),
                        )
                    nc.vector.tensor_copy(ys[:, oc, :], yp[:])
                nc.gpsimd.dma_start(y_d[ss, :], ys[:])

    nc.compile()
    return nc


def _get_nc():
    if "nc" not in _NC_CACHE:
        _NC_CACHE["nc"] = _build_nc()
    return _NC_CACHE["nc"]


def kernel(**inputs):
    from concourse import bass_utils

    q, k, v = inputs["query"], inputs["key"], inputs["value"]
    Wq, Wk, Wv, Wo = inputs["Wq"], inputs["Wk"], inputs["Wv"], inputs["Wo"]
    bq, bk, bv, bo = inputs["bq"], inputs["bk"], inputs["bv"], inputs["bo"]

    nc = _get_nc()
    in_maps = []
    for c in range(NCORES):
        b, hg = divmod(c, 2)
        r0 = hg * DG
        rs = slice(r0, r0 + DG)
        in_maps.append(
            {
                "xqT": np.ascontiguousarray(q[b].T),
                "xkT": np.ascontiguousarray(k[b].T),
                "xvT": np.ascontiguousarray(v[b].T),
                "wqT": np.ascontiguousarray(Wq[rs, :].T),
                "wkT": np.ascontiguousarray(Wk[rs, :].T),
                "wvT": np.ascontiguousarray(Wv[rs, :].T),
                "woT": np.ascontiguousarray(Wo[:, rs].T),
                "bqc": np.ascontiguousarray(bq[rs].reshape(4, 128).T),
                "bkc": np.ascontiguousarray(bk[rs].reshape(4, 128).T),
                "bvr": np.ascontiguousarray(bv[rs]),
            }
        )
    import os

    trace = bool(os.environ.get("KERNEL_TRACE"))
    res = bass_utils.run_bass_kernel_spmd(
        nc, in_maps, core_ids=list(range(NCORES)), trace=trace
    )
    global LAST_EXEC_NS
    LAST_EXEC_NS = res.exec_time_ns
    out = np.empty((B, S, D), np.float32)
    for b in range(B):
        out[b] = res.results[2 * b]["y"] + res.results[2 * b + 1]["y"] + bo[None, :]
    return out


# revision 7
# speedup vs baseline: 1.1900x; 1.1900x over previous
import numpy as np

# nn_MultiHeadedAttention: B=4, S=2048, D_MODEL=1024, H=16, D_K=64, fp32.
# Sharding: 8 cores = 4 batches x 2 head-groups (8 heads each).
# Per-core: head-group projections (fp32r matmuls), scores computed
# transposed ST[k,q], exp on ACT straight out of 2-bank PSUM reads,
# PV with an appended ones-column so the softmax denominator falls out
# of row 64 of the PV accumulation, PE-broadcast reciprocal, out-proj
# partial y; host sums the two head-group partials and adds bo.

B, S, D, H, DK = 4, 2048, 1024, 16, 64
NCORES = 8
DG = 512  # dims per head-group (8 heads x 64)

_NC_CACHE = {}
LAST_EXEC_NS = None


def _build_nc():
    import concourse.bacc as bacc
    import concourse.tile as tile
    from concourse import mybir

    F32R = mybir.dt.float32r
    F32 = mybir.dt.float32
    EXP = mybir.ActivationFunctionType.Exp

    nc = bacc.Bacc(None, target_bir_lowering=False, debug=True)

    xqT = nc.dram_tensor("xqT", [D, S], F32R, kind="ExternalInput")
    xkT = nc.dram_tensor("xkT", [D, S], F32R, kind="ExternalInput")
    xvT = nc.dram_tensor("xvT", [D, S], F32R, kind="ExternalInput")
    wqT = nc.dram_tensor("wqT", [D, DG], F32R, kind="ExternalInput")
    wkT = nc.dram_tensor("wkT", [D, DG], F32R, kind="ExternalInput")
    wvT = nc.dram_tensor("wvT", [D, DG], F32R, kind="ExternalInput")
    woT = nc.dram_tensor("woT", [DG, D], F32R, kind="ExternalInput")
    bqc = nc.dram_tensor("bqc", [128, 4], F32, kind="ExternalInput")
    bkc = nc.dram_tensor("bkc", [128, 4], F32, kind="ExternalInput")
    bvr = nc.dram_tensor("bvr", [DG], F32, kind="ExternalInput")
    y_d = nc.dram_tensor("y", [S, D], F32R, kind="ExternalOutput")

    import concourse.bass as bass

    with (
        tile.TileContext(nc) as tc,
        nc.allow_low_precision(reason="float32r carries full fp32 bits"),
        tc.tile_pool(name="persist", bufs=1) as persist,
    ):
        QT = [persist.tile([128, S], F32R, name=f"QT{p}") for p in range(4)]
        KT = [persist.tile([128, S], F32R, name=f"KT{p}") for p in range(4)]
        AT = [persist.tile([128, S], F32R, name=f"AT{p}") for p in range(4)]
        vones = [persist.tile([128, 8, 65], F32R, name=f"vones{s}") for s in range(16)]
        bq_sb = persist.tile([128, 4], F32, name="bq_sb")
        bk_sb = persist.tile([128, 4], F32, name="bk_sb")
        bv_sb = persist.tile([128, DG], F32, name="bv_sb")
        ones_row = persist.tile([1, 64], F32R, name="ones_row")

        nc.gpsimd.dma_start(bq_sb[:], bqc[:])
        nc.gpsimd.dma_start(bk_sb[:], bkc[:])
        bv_ap = bvr[:]
        bv_bcast = bass.AP(tensor=bv_ap.tensor, offset=bv_ap.offset, ap=[[0, 128], *bv_ap.ap])
        nc.gpsimd.dma_start(bv_sb[:], bv_bcast)
        nc.vector.memset(ones_row[:].bitcast(F32), 1.0)
        for s in range(16):
            nc.vector.memset(vones[s][:, :, 64:65].bitcast(F32), 1.0)

        # ---- Q then K projections ----
        for (x_d, w_d, b_sb, OUT) in ((xqT, wqT, bq_sb, QT), (xkT, wkT, bk_sb, KT)):
            with (
                tc.tile_pool(name="projqk", bufs=2) as stage,
                tc.tile_pool(name="psprojqk", bufs=4, space="PSUM") as psp,
            ):
                wt = stage.tile([128, 8, DG], F32R, name="wt", bufs=1)
                for i in range(8):
                    nc.gpsimd.dma_start(wt[:, i, :], w_d[i * 128 : (i + 1) * 128, :])
                for qc in range(4):
                    qs = slice(qc * 512, (qc + 1) * 512)
                    xs = stage.tile([128, 8, 512], F32R, name="xs")
                    for i in range(8):
                        eng = nc.sync if i % 2 == 0 else nc.scalar
                        eng.dma_start(xs[:, i, :], x_d[i * 128 : (i + 1) * 128, qs])
                    for p in range(4):
                        pp = psp.tile([128, 512], F32, name="pp")
                        for i in range(8):
                            nc.tensor.matmul(
                                pp[:],
                                wt[:, i, p * 128 : (p + 1) * 128],
                                xs[:, i, :],
                                start=(i == 0),
                                stop=(i == 7),
                            )
                        nc.vector.tensor_scalar_add(OUT[p][:, qs], pp[:], b_sb[:, p : p + 1])

        # ---- V projection (natural layout [s, d] + ones column) ----
        with (
            tc.tile_pool(name="projv", bufs=3) as stage,
            tc.tile_pool(name="psprojv", bufs=4, space="PSUM") as psp,
        ):
            wv = stage.tile([128, 8, DG], F32R, name="wv", bufs=1)
            for i in range(8):
                nc.gpsimd.dma_start(wv[:, i, :], wvT[i * 128 : (i + 1) * 128, :])
            for sb in range(16):
                ss = slice(sb * 128, (sb + 1) * 128)
                xv = stage.tile([128, 8, 128], F32R, name="xv")
                for i in range(8):
                    eng = nc.sync if i % 2 == 0 else nc.scalar
                    eng.dma_start(xv[:, i, :], xvT[i * 128 : (i + 1) * 128, ss])
                vp = psp.tile([128, DG], F32, name="vp")
                for i in range(8):
                    nc.tensor.matmul(
                        vp[:], xv[:, i, :], wv[:, i, :], start=(i == 0), stop=(i == 7)
                    )
                nc.vector.tensor_add(
                    vones[sb][:, :, 0:64],
                    vp[:].rearrange("p (h d) -> p h d", h=8),
                    bv_sb[:].rearrange("p (h d) -> p h d", h=8),
                )

        # ---- attention: scores -> exp -> PV(+denom) -> normalize ----
        # Per (p, qc): score matmuls packed as head pairs (tile rows 0/64),
        # exp staged to SBUF chunks, PV in uninterrupted runs of 8 per head,
        # normalization software-pipelined one iteration behind.
        with (
            tc.tile_pool(name="attn_sb", bufs=3) as asb,
            tc.tile_pool(name="ps_st", bufs=2, space="PSUM") as ps_st,
            tc.tile_pool(name="ps_pv", bufs=2, space="PSUM") as ps_pv,
        ):
            def emit_norm(state):
                pp, pqs, ppv, prec = state
                for h in range(2):
                    hb = h * 64
                    bc = ps_st.tile([128, 2, 512], F32, name="st")
                    nc.tensor.matmul(
                        bc[0:64, 0, :], ones_row[:], prec[h][:], start=True, stop=True
                    )
                    bcs = asb.tile([64, 512], F32R, name="bcs", bufs=1)
                    nc.vector.tensor_copy(bcs[:], bc[0:64, 0, :])
                    nc.vector.tensor_mul(AT[pp][hb : hb + 64, pqs], ppv[h][0:64, :], bcs[:])

            prev = None
            for p in range(4):
                for qc in range(4):
                    qs = slice(qc * 512, (qc + 1) * 512)
                    pv = [ps_pv.tile([128, 512], F32, name=f"pv{h}") for h in range(2)]
                    ech_tiles = []

                    def sc_chunk(c):
                        ech = asb.tile([128, 2, 4, 512], F32R, name="ech", bufs=3)
                        ech_tiles.append(ech)
                        for kbi in range(4):
                            kb = c * 4 + kbi
                            st = ps_st.tile([128, 2, 512], F32, name="st")
                            for h in range(2):
                                nc.tensor.matmul(
                                    st[:, h, :],
                                    KT[p][h * 64 : (h + 1) * 64, kb * 128 : (kb + 1) * 128],
                                    QT[p][h * 64 : (h + 1) * 64, qs],
                                    start=True,
                                    stop=True,
                                )
                            nc.scalar.activation(
                                out=ech[:, :, kbi, :], in_=st[:], func=EXP, scale=0.125
                            )

                    def pv_half(half):
                        for h in range(2):
                            hidx = p * 2 + h
                            for kbj in range(8):
                                c, kbi = divmod(kbj, 4)
                                kb = half * 8 + kbj
                                nc.tensor.matmul(
                                    pv[h][0:65, :],
                                    vones[kb][:, hidx, :],
                                    ech_tiles[half * 2 + c][:, h, kbi, :],
                                    start=(half == 0 and kbj == 0),
                                    stop=(half == 1 and kbj == 7),
                                )

                    sc_chunk(0)
                    sc_chunk(1)
                    sc_chunk(2)
                    pv_half(0)
                    sc_chunk(3)
                    pv_half(1)
                    if prev is not None:
                        emit_norm(prev)
                    rec = []
                    for h in range(2):
                        r = asb.tile([1, 512], F32R, name=f"rec{h}", bufs=2)
                        nc.vector.reciprocal(r[:], pv[h][64:65, :])
                        rec.append(r)
                    prev = (p, qs, pv, rec)
            emit_norm(prev)

        # ---- output projection (partial y for this head-group) ----
        with (
            tc.tile_pool(name="out_sb", bufs=1) as osb,
            tc.tile_pool(name="out_y", bufs=3) as oy,
            tc.tile_pool(name="ps_y", bufs=4, space="PSUM") as ps_y,
        ):
            wo = osb.tile([128, 4, D], F32R, name="wo")
            for p in range(4):
                nc.gpsimd.dma_start(wo[:, p, :], woT[p * 128 : (p + 1) * 128, :])
            for sb in range(16):
                ss = slice(sb * 128, (sb + 1) * 128)
                ys = oy.tile([128, 2, 512], F32R, name="ys")
                for oc in range(2):
                    yp = ps_y.tile([128, 512], F32, name="yp")
                    for p in range(4):
                        nc.tensor.matmul(
                            yp[:],
                            AT[p][:, ss],
                            wo[:, p, oc * 512 : (oc + 1) * 512],
                            start=(p == 0),
                            stop=(p == 3),
                        )
                    nc.vector.tensor_copy(ys[:, oc, :], yp[:])
                nc.gpsimd.dma_start(y_d[ss, :], ys[:])

    nc.compile()
    return nc


def _get_nc():
    if "nc" not in _NC_CACHE:
        _NC_CACHE["nc"] = _build_nc()
    return _NC_CACHE["nc"]


def kernel(**inputs):
    from concourse import bass_utils

    q, k, v = inputs["query"], inputs["key"], inputs["value"]
    Wq, Wk, Wv, Wo = inputs["Wq"], inputs["Wk"], inputs["Wv"], inputs["Wo"]
    bq, bk, bv, bo = inputs["bq"], inputs["bk"], inputs["bv"], inputs["bo"]

    nc = _get_nc()
    in_maps = []
    for c in range(NCORES):
        b, hg = divmod(c, 2)
        r0 = hg * DG
        rs = slice(r0, r0 + DG)
        in_maps.append(
            {
                "xqT": np.ascontiguousarray(q[b].T),
                "xkT": np.ascontiguousarray(k[b].T),
                "xvT": np.ascontiguousarray(v[b].T),
                "wqT": np.ascontiguousarray(Wq[rs, :].T),
                "wkT": np.ascontiguousarray(Wk[rs, :].T),
                "wvT": np.ascontiguousarray(Wv[rs, :].T),
                "woT": np.ascontiguousarray(Wo[:, rs].T),
                "bqc": np.ascontiguousarray(bq[rs].reshape(4, 128).T),
                "bkc": np.ascontiguousarray(bk[rs].reshape(4, 128).T),
                "bvr": np.ascontiguousarray(bv[rs]),
            }
        )
    import os

    trace = bool(os.environ.get("KERNEL_TRACE"))
    res = bass_utils.run_bass_kernel_spmd(
        nc, in_maps, core_ids=list(range(NCORES)), trace=trace
    )
    global LAST_EXEC_NS
    LAST_EXEC_NS = res.exec_time_ns
    out = np.empty((B, S, D), np.float32)
    for b in range(B):
        out[b] = res.results[2 * b]["y"] + res.results[2 * b + 1]["y"] + bo[None, :]
    return out


# revision 13
# speedup vs baseline: 1.7323x; 1.4557x over previous
import numpy as np

# nn_MultiHeadedAttention: B=4, S=2048, D_MODEL=1024, H=16, D_K=64, fp32.
# Sharding: 8 cores = 4 batches x 2 head-groups (8 heads each).
# Per-core: head-group projections (fp32r matmuls), scores computed
# transposed ST[k,q], exp on ACT straight out of 2-bank PSUM reads,
# PV with an appended ones-column so the softmax denominator falls out
# of row 64 of the PV accumulation, PE-broadcast reciprocal, out-proj
# partial y; host sums the two head-group partials and adds bo.

B, S, D, H, DK = 4, 2048, 1024, 16, 64
NCORES = 8
DG = 512  # dims per head-group (8 heads x 64)

_NC_CACHE = {}
LAST_EXEC_NS = None


def _build_nc():
    import concourse.bacc as bacc
    import concourse.tile as tile
    from concourse import mybir

    F32R = mybir.dt.float32r
    F32 = mybir.dt.float32
    EXP = mybir.ActivationFunctionType.Exp

    nc = bacc.Bacc(None, target_bir_lowering=False, debug=True)

    xqT = nc.dram_tensor("xqT", [D, S], F32R, kind="ExternalInput")
    xkT = nc.dram_tensor("xkT", [D, S], F32R, kind="ExternalInput")
    xvT = nc.dram_tensor("xvT", [D, S], F32R, kind="ExternalInput")
    wqT = nc.dram_tensor("wqT", [D, DG], F32R, kind="ExternalInput")
    wkT = nc.dram_tensor("wkT", [D, DG], F32R, kind="ExternalInput")
    wvT = nc.dram_tensor("wvT", [D, DG], F32R, kind="ExternalInput")
    woT = nc.dram_tensor("woT", [DG, D], F32R, kind="ExternalInput")
    bqc = nc.dram_tensor("bqc", [128, 4], F32, kind="ExternalInput")
    bkc = nc.dram_tensor("bkc", [128, 4], F32, kind="ExternalInput")
    bvr = nc.dram_tensor("bvr", [DG], F32, kind="ExternalInput")
    y_d = nc.dram_tensor("y", [S, D], F32R, kind="ExternalOutput")

    import concourse.bass as bass

    with (
        tile.TileContext(nc) as tc,
        nc.allow_low_precision(reason="float32r carries full fp32 bits"),
        tc.tile_pool(name="persist", bufs=1) as persist,
    ):
        QT = [persist.tile([128, S], F32R, name=f"QT{p}") for p in range(4)]
        KT = [persist.tile([128, S], F32R, name=f"KT{p}") for p in range(4)]
        AT = [persist.tile([128, S], F32R, name=f"AT{p}") for p in range(4)]
        vones = [persist.tile([128, 8, 65], F32R, name=f"vones{s}") for s in range(16)]
        bq_sb = persist.tile([128, 4], F32, name="bq_sb")
        bk_sb = persist.tile([128, 4], F32, name="bk_sb")
        bv_sb = persist.tile([128, DG], F32, name="bv_sb")
        ones_row = persist.tile([1, 64], F32R, name="ones_row")

        nc.gpsimd.dma_start(bq_sb[:], bqc[:])
        nc.gpsimd.dma_start(bk_sb[:], bkc[:])
        bv_ap = bvr[:]
        bv_bcast = bass.AP(tensor=bv_ap.tensor, offset=bv_ap.offset, ap=[[0, 128], *bv_ap.ap])
        nc.gpsimd.dma_start(bv_sb[:], bv_bcast)
        nc.vector.memset(ones_row[:].bitcast(F32), 1.0)
        for s in range(16):
            nc.vector.memset(vones[s][:, :, 64:65].bitcast(F32), 1.0)

        # ---- Q, K, V projections (one scope; weight arena rotates so the
        # next phase's weights stream in during the current phase) ----
        with (
            tc.tile_pool(name="proj", bufs=2) as stage,
            tc.tile_pool(name="psproj", bufs=4, space="PSUM") as psp,
        ):
            def load_w(w_d):
                wt = stage.tile([128, 8, DG], F32R, name="wt")
                for i in range(8):
                    nc.gpsimd.dma_start(wt[:, i, :], w_d[i * 128 : (i + 1) * 128, :])
                return wt

            def qk_phase(x_d, b_sb, wt, OUT):
                for qc in range(4):
                    qs = slice(qc * 512, (qc + 1) * 512)
                    xs = stage.tile([128, 8, 512], F32R, name="xs")
                    for i in range(8):
                        eng = nc.sync if i % 2 == 0 else nc.scalar
                        eng.dma_start(xs[:, i, :], x_d[i * 128 : (i + 1) * 128, qs])
                    for p in range(4):
                        pp = psp.tile([128, 512], F32, name="pp")
                        for i in range(8):
                            nc.tensor.matmul(
                                pp[:],
                                wt[:, i, p * 128 : (p + 1) * 128],
                                xs[:, i, :],
                                start=(i == 0),
                                stop=(i == 7),
                            )
                        nc.vector.tensor_scalar_add(OUT[p][:, qs], pp[:], b_sb[:, p : p + 1])

            wq = load_w(wqT)
            wk = load_w(wkT)
            qk_phase(xqT, bq_sb, wq, QT)
            wv = load_w(wvT)  # reuses wq's buffer; WAR on Q reads already recorded
            qk_phase(xkT, bk_sb, wk, KT)
            # V projection (natural layout [s, d] + ones column)
            for sb in range(16):
                ss = slice(sb * 128, (sb + 1) * 128)
                xv = stage.tile([128, 8, 128], F32R, name="xv", bufs=3)
                for i in range(8):
                    eng = nc.sync if i % 2 == 0 else nc.scalar
                    eng.dma_start(xv[:, i, :], xvT[i * 128 : (i + 1) * 128, ss])
                vp = psp.tile([128, DG], F32, name="vp")
                for i in range(8):
                    nc.tensor.matmul(
                        vp[:], xv[:, i, :], wv[:, i, :], start=(i == 0), stop=(i == 7)
                    )
                nc.vector.tensor_add(
                    vones[sb][:, :, 0:64],
                    vp[:].rearrange("p (h d) -> p h d", h=8),
                    bv_sb[:].rearrange("p (h d) -> p h d", h=8),
                )

        # ---- attention: scores -> exp -> PV(+denom) -> normalize ----
        # Per (p, qc): score matmuls packed as head pairs (tile rows 0/64),
        # exp staged to SBUF chunks, PV in uninterrupted runs of 8 per head,
        # normalization software-pipelined one iteration behind.
        with (
            tc.tile_pool(name="attn_sb", bufs=3) as asb,
            tc.tile_pool(name="ps_st", bufs=2, space="PSUM") as ps_st,
            tc.tile_pool(name="ps_pv", bufs=2, space="PSUM") as ps_pv,
        ):
            def emit_norm(state):
                pp, pqs, ppv, prec = state
                bc = [ps_st.tile([128, 2, 512], F32, name="st") for _ in range(2)]
                for h in range(2):
                    nc.tensor.matmul(
                        bc[h][0:64, 0, :], ones_row[:], prec[h][:], start=True, stop=True
                    )
                bcs = [asb.tile([64, 512], F32R, name=f"bcs{h}", bufs=1) for h in range(2)]
                for h in range(2):
                    nc.vector.tensor_copy(bcs[h][:], bc[h][0:64, 0, :])
                for h in range(2):
                    hb = h * 64
                    nc.vector.tensor_mul(
                        AT[pp][hb : hb + 64, pqs], ppv[h][0:64, :], bcs[h][:]
                    )

            prev = None
            for p in range(4):
                for qc in range(4):
                    qs = slice(qc * 512, (qc + 1) * 512)
                    pv = [ps_pv.tile([128, 512], F32, name=f"pv{h}") for h in range(2)]
                    ech_tiles = []

                    def sc_chunk(c):
                        ech = asb.tile([128, 2, 4, 512], F32R, name="ech", bufs=3)
                        ech_tiles.append(ech)
                        for kbi in range(4):
                            kb = c * 4 + kbi
                            st = ps_st.tile([128, 2, 512], F32, name="st")
                            for h in range(2):
                                nc.tensor.matmul(
                                    st[:, h, :],
                                    KT[p][h * 64 : (h + 1) * 64, kb * 128 : (kb + 1) * 128],
                                    QT[p][h * 64 : (h + 1) * 64, qs],
                                    start=True,
                                    stop=True,
                                )
                            nc.scalar.activation(
                                out=ech[:, :, kbi, :], in_=st[:], func=EXP, scale=0.125
                            )

                    def pv_half(half):
                        for h in range(2):
                            hidx = p * 2 + h
                            for kbj in range(8):
                                c, kbi = divmod(kbj, 4)
                                kb = half * 8 + kbj
                                nc.tensor.matmul(
                                    pv[h][0:65, :],
                                    vones[kb][:, hidx, :],
                                    ech_tiles[half * 2 + c][:, h, kbi, :],
                                    start=(half == 0 and kbj == 0),
                                    stop=(half == 1 and kbj == 7),
                                )

                    sc_chunk(0)
                    sc_chunk(1)
                    sc_chunk(2)
                    pv_half(0)
                    if prev is not None:
                        emit_norm(prev)
                        prev = None
                    sc_chunk(3)
                    pv_half(1)
                    rec = []
                    for h in range(2):
                        r = asb.tile([1, 512], F32R, name=f"rec{h}", bufs=2)
                        nc.vector.reciprocal(r[:], pv[h][64:65, :])
                        rec.append(r)
                    prev = (p, qs, pv, rec)
            emit_norm(prev)

        # ---- output projection (partial y for this head-group) ----
        with (
            tc.tile_pool(name="out_sb", bufs=1) as osb,
            tc.tile_pool(name="out_y", bufs=3) as oy,
            tc.tile_pool(name="ps_y", bufs=4, space="PSUM") as ps_y,
        ):
            wo = osb.tile([128, 4, D], F32R, name="wo")
            for p in range(4):
                nc.gpsimd.dma_start(wo[:, p, :], woT[p * 128 : (p + 1) * 128, :])
            for sb in range(16):
                ss = slice(sb * 128, (sb + 1) * 128)
                ys = oy.tile([128, 2, 512], F32R, name="ys")
                for oc in range(2):
                    yp = ps_y.tile([128, 512], F32, name="yp")
                    for p in range(4):
                        nc.tensor.matmul(
                            yp[:],
                            AT[p][:, ss],
                            wo[:, p, oc * 512 : (oc + 1) * 512],
                            start=(p == 0),
                            stop=(p == 3),
                        )
                    nc.vector.tensor_copy(ys[:, oc, :], yp[:])
                nc.gpsimd.dma_start(y_d[ss, :], ys[:])

    nc.compile()
    return nc


def _get_nc():
    if "nc" not in _NC_CACHE:
        _NC_CACHE["nc"] = _build_nc()
    return _NC_CACHE["nc"]


def kernel(**inputs):
    from concourse import bass_utils

    q, k, v = inputs["query"], inputs["key"], inputs["value"]
    Wq, Wk, Wv, Wo = inputs["Wq"], inputs["Wk"], inputs["Wv"], inputs["Wo"]
    bq, bk, bv, bo = inputs["bq"], inputs["bk"], inputs["bv"], inputs["bo"]

    nc = _get_nc()
    in_maps = []
    for c in range(NCORES):
        b, hg = divmod(c, 2)
        r0 = hg * DG
        rs = slice(r0, r0 + DG)
        in_maps.append(
            {
                "xqT": np.ascontiguousarray(q[b].T),
                "xkT": np.ascontiguousarray(k[b].T),
                "xvT": np.ascontiguousarray(v[b].T),
                "wqT": np.ascontiguousarray(Wq[rs, :].T),
                "wkT": np.ascontiguousarray(Wk[rs, :].T),
                "wvT": np.ascontiguousarray(Wv[rs, :].T),
                "woT": np.ascontiguousarray(Wo[:, rs].T),
                "bqc": np.ascontiguousarray(bq[rs].reshape(4, 128).T),
                "bkc": np.ascontiguousarray(bk[rs].reshape(4, 128).T),
                "bvr": np.ascontiguousarray(bv[rs]),
            }
        )
    import os

    trace = bool(os.environ.get("KERNEL_TRACE"))
    res = bass_utils.run_bass_kernel_spmd(
        nc, in_maps, core_ids=list(range(NCORES)), trace=trace
    )
    global LAST_EXEC_NS
    LAST_EXEC_NS = res.exec_time_ns
    out = np.empty((B, S, D), np.float32)
    for b in range(B):
        out[b] = res.results[2 * b]["y"] + res.results[2 * b + 1]["y"] + bo[None, :]
    return out
